# revision 1
# baseline (speedup 1.0000x reference)
"""Trainium2 Bass kernel for the spike-decoder GNN message-passing module.

Math (per batch b, output time tau in [0, T-2], variable v):
  out[b,tau,v] = bias[v]
               + sum_{i,k} w[v,i,k] * x[b,i,tau+k-(K-2)]          (static conv)
               + sum_{e: recv[e]=v} sum_k dw[e,b,tau,k] * x[b,send[e],tau+k-(K-2)]
with w = conv_weight masked at w[i,i,K-1] = 0, x = spikes[...,0] transposed to
[b, nvar, t], and out-of-range x treated as zero.

Sharding: 8 cores = (b in 0..3) x (time half h in 0..1). Each core computes a
1024-wide tau window ([0,1024) or [1023,2047) — one overlapping column keeps
shapes uniform for SPMD). dyn_weights is the only big tensor (268 MB); its
[E, 1024, K] slice per core is the memory-bound stream.

On-core algorithm (all fp32):
  - xg[e,:] = x[send[e],:] gathered via one-hot matmul on PE (exact: x is 0/1)
  - products P[e,(tau,k)] = dw_tile * sliding-window(xg) on DVE (one big
    tensor_tensor per e-tile with an overlapping stride-1 AP for the window)
  - k-reduction + recv-scatter + transpose folded into PE: for each k, a
    matmul with stationary one-hot recv matrix and moving operand = strided
    columns P[:, tau*K+k], accumulating into PSUM[v, tau]
  - static conv: 16 matmuls with stationary wT_k and shifted xpad slices
  - bias: rank-1 matmul (bias x ones)
All terms accumulate into one PSUM bank [v, 512], copied out by ScalarE.
Output is [v, tau] per core; host transposes while assembling the result.
"""

import numpy as np

B, T, NVAR, K, E = 4, 2048, 128, 16, 512
TAU = T - 1            # 2047
L = 1024               # per-core tau window
NC_COUNT = 8
W_XPAD = L + K         # 1040 (1039 used; padded even for f32r matmul ISA)
ETILES = E // 128      # 4
CHUNK = 512            # tau chunk per PSUM bank
NCHUNK = L // CHUNK    # 2

_PROGRAM = None


def _build_program():
    import concourse.bass as bass
    import concourse.bacc as bacc
    import concourse.mybir as mybir
    import concourse.tile as tile

    f32 = mybir.dt.float32
    # float32r: same fp32 bytes, but the PE streams 1 row/cycle (vs 4 for
    # strict fp32 which needs 2 half-rate passes) when the moving dim >= 256.
    f32r = mybir.dt.float32r
    bf16 = mybir.dt.bfloat16
    # Bacc (not plain Bass): its compile pipeline runs generate_event_semaphores,
    # which splits multi-semaphore waits — a raw fp32 Matmult supports only one
    # sync-wait slot and walrus rejects more ("Too many sync wait commands").
    nc = bacc.Bacc()

    xpad_d = nc.declare_dram_parameter("xpad", [NVAR, W_XPAD], f32r, isOutput=False)
    dw_d = nc.declare_dram_parameter("dw", [NCHUNK * E, CHUNK * K], f32, isOutput=False)
    ssend_d = nc.declare_dram_parameter("ssend", [NVAR, E], f32r, isOutput=False)
    wt_d = nc.declare_dram_parameter("wt", [NVAR, K * NVAR], f32r, isOutput=False)
    recv_d = nc.declare_dram_parameter("recvT", [128, ETILES * NVAR], bf16, isOutput=False)
    bo_d = nc.declare_dram_parameter("bias_ones", [1, NVAR + CHUNK], f32r, isOutput=False)
    y_d = nc.declare_dram_parameter("yT", [NVAR, L], f32, isOutput=True)

    with tile.TileContext(nc) as tc:
        with (
            tc.tile_pool(name="consts", bufs=1) as consts,
            tc.tile_pool(name="xgp", bufs=1) as xgp,
            tc.tile_pool(name="gpsum", bufs=2, space=bass.MemorySpace.PSUM) as gpsum,
            tc.tile_pool(name="dwp", bufs=3) as dwp,
            tc.tile_pool(name="prodp", bufs=3) as prodp,
            tc.tile_pool(name="opsum", bufs=2, space=bass.MemorySpace.PSUM) as opsum,
            tc.tile_pool(name="resp", bufs=2) as resp,
        ):
            NT = NCHUNK * ETILES  # 8 dw tiles
            HK = CHUNK * K // 2   # half-tile product columns (4096)
            HC = CHUNK // 2       # tau columns per half (256)

            # SP/HWDGE issue order = completion order (per-engine FIFO):
            # gather inputs first (small), then the dw stream owns the queue.
            # Tiles 0 and 7 are split into half-DMAs (16KB packets, slightly
            # slower) so the first multiply starts ~5us earlier and the tail
            # half overlaps its matmuls; middle tiles stay whole for peak
            # 32KB-packet bandwidth.
            xpad = consts.tile([NVAR, W_XPAD], f32r)
            nc.sync.dma_start(xpad[:], xpad_d[:])
            ssend = consts.tile([NVAR, E], f32r)
            nc.sync.dma_start(ssend[:], ssend_d[:])

            def dw_dma(dwt, ti, halves):
                h2, et = divmod(ti, ETILES)
                r0 = h2 * E + et * 128
                if halves:
                    for half in range(2):
                        nc.sync.dma_start(
                            dwt[:, half * HK:(half + 1) * HK],
                            dw_d[r0:r0 + 128, half * HK:(half + 1) * HK],
                        )
                else:
                    nc.sync.dma_start(dwt[:], dw_d[r0:r0 + 128, :])

            dwt_tiles = []
            for ti in range(NT):
                dwt = dwp.tile([128, CHUNK * K], f32, name="dwt", tag="dwt")
                dwt_tiles.append(dwt)
            dw_dma(dwt_tiles[0], 0, halves=True)
            # remaining small constants slot in behind the first dw tile
            wt = consts.tile([NVAR, K * NVAR], f32r)
            nc.sync.dma_start(wt[:], wt_d[:])
            recvT = consts.tile([128, ETILES * NVAR], bf16)
            nc.sync.dma_start(recvT[:], recv_d[:])
            bias_ones = consts.tile([1, NVAR + CHUNK], f32r)
            nc.sync.dma_start(bias_ones[:], bo_d[:])
            for ti in range(1, NT):
                dw_dma(dwt_tiles[ti], ti, halves=(ti == NT - 1))

            # Gather sender rows: xg[et][p, j] = xpad[send[et*128+p], j]
            xg = []
            for et in range(ETILES):
                xgt = xgp.tile([128, W_XPAD], f32, name=f"xg{et}", tag=f"xg{et}")
                for j0 in range(0, W_XPAD, CHUNK):
                    jw = min(CHUNK, W_XPAD - j0)
                    gps = gpsum.tile([128, CHUNK], f32, name="gps", tag="gps")
                    nc.tensor.matmul(
                        gps[:, :jw],
                        ssend[:, et * 128:(et + 1) * 128],
                        xpad[:, j0:j0 + jw],
                        start=True, stop=True,
                    )
                    nc.scalar.copy(xgt[:, j0:j0 + jw], gps[:, :jw])
                xg.append(xgt)

            ops_tiles = []
            for h2 in range(NCHUNK):
                o = opsum.tile([128, CHUNK], f32, name=f"ops{h2}", tag=f"ops{h2}")
                ops_tiles.append(o)

            def static_mm(h2, k, start=False):
                t0 = h2 * CHUNK
                nc.tensor.matmul(
                    ops_tiles[h2][:],
                    wt[:, k * NVAR:(k + 1) * NVAR],
                    xpad[:, t0 + k:t0 + k + CHUNK],
                    start=start, stop=False,
                )

            def bias_mm(h2):
                nc.tensor.matmul(
                    ops_tiles[h2][:],
                    bias_ones[:1, 0:NVAR],
                    bias_ones[:1, NVAR:NVAR + CHUNK],
                    start=False, stop=False,
                )

            # chunk-0 static conv + bias up front (PE warmup while dw streams)
            for k in range(K):
                static_mm(0, k, start=(k == 0))
            bias_mm(0)

            # chunk-1 static matmuls fill PE gaps across the first 7 groups
            fill = [("s", k) for k in range(K)] + [("b", None)]
            fills_per_group = [3, 3, 3, 2, 2, 2, 2, 0]

            KH = K // 2
            for ti in range(NT):
                h2, et = divmod(ti, ETILES)
                t0 = h2 * CHUNK
                dwt = dwt_tiles[ti]
                pt = prodp.tile([128, CHUNK * K], bf16, name="pt", tag="pt")
                drow = dwt.tensor.shape[-1]
                prow = pt.tensor.shape[-1]
                xrow = xg[et].tensor.shape[-1]
                # dw arrives k-major: dwt[e, k*CHUNK + tau]. Products keep that
                # layout, so every AP below is stride-1 in its innermost dim
                # (strided PE moving operands cost ~3-6 cycles/column, and
                # strided bf16 DVE writes hit sub-word read-modify-write).
                # Each tile is processed as two k-halves: the 8 matmuls of
                # half a run while DVE multiplies half b.
                for half in range(2):
                    k0 = half * KH
                    in0 = bass.AP(dwt.tensor, k0 * CHUNK,
                                  [[drow, 128], [CHUNK, KH], [1, CHUNK]])
                    # sliding window: in1[p, k, tau] = xg[p, t0 + tau + k]
                    in1 = bass.AP(xg[et].tensor, t0 + k0,
                                  [[xrow, 128], [1, KH], [1, CHUNK]])
                    out3 = bass.AP(pt.tensor, k0 * CHUNK,
                                   [[prow, 128], [CHUNK, KH], [1, CHUNK]])
                    nc.vector.tensor_mul(out3, in0, in1)
                    # k-reduction + recv scatter on PE (bf16, contiguous rhs):
                    # psum[v, tau] += sum_e recvT[e, v] * P[e, k*CHUNK + tau]
                    for k in range(k0, k0 + KH):
                        rhs = bass.AP(pt.tensor, k * CHUNK,
                                      [[prow, 128], [1, CHUNK]])
                        nc.tensor.matmul(
                            ops_tiles[h2][:],
                            recvT[:, et * NVAR:(et + 1) * NVAR],
                            rhs,
                            start=False,
                            stop=(et == ETILES - 1 and k == K - 1),
                        )
                for _ in range(fills_per_group[ti]):
                    kind, k = fill.pop(0)
                    if kind == "s":
                        static_mm(1, k, start=(k == 0))
                    else:
                        bias_mm(1)
                if et == ETILES - 1:
                    res = resp.tile([128, CHUNK], f32, name="res", tag="res")
                    nc.scalar.copy(res[:], ops_tiles[h2][:])
                    nc.gpsimd.dma_start(y_d[:, t0:t0 + CHUNK], res[:])

    nc.compile()
    return nc


def _get_program():
    global _PROGRAM
    if _PROGRAM is None:
        _PROGRAM = _build_program()
    return _PROGRAM


def _host_prep(spikes, conv_weight, conv_bias, dyn_weights, edge_send, edge_recv):
    spikes = np.asarray(spikes, dtype=np.float32)
    conv_weight = np.asarray(conv_weight, dtype=np.float32)
    conv_bias = np.asarray(conv_bias, dtype=np.float32)
    dyn_weights = np.asarray(dyn_weights, dtype=np.float32)
    edge_send = np.asarray(edge_send, dtype=np.int64)
    edge_recv = np.asarray(edge_recv, dtype=np.int64)

    x = np.ascontiguousarray(spikes[..., 0].transpose(0, 2, 1))  # [B, NVAR, T]

    ssend = np.zeros((NVAR, E), np.float32)
    ssend[edge_send, np.arange(E)] = 1.0

    import ml_dtypes
    recvT = np.zeros((128, ETILES * NVAR), ml_dtypes.bfloat16)
    for et in range(ETILES):
        rr = edge_recv[et * 128:(et + 1) * 128]
        recvT[np.arange(128), et * NVAR + rr] = 1.0

    w = conv_weight.copy()
    w[np.arange(NVAR), np.arange(NVAR), K - 1] = 0.0
    wt = np.ascontiguousarray(w.transpose(1, 2, 0)).reshape(NVAR, K * NVAR)

    bias_ones = np.concatenate(
        [conv_bias, np.ones(CHUNK, np.float32)]
    ).reshape(1, NVAR + CHUNK).astype(np.float32)

    in_maps = []
    for core in range(NC_COUNT):
        b, h = divmod(core, 2)
        tau0 = 0 if h == 0 else TAU - L  # 0 or 1023
        xpad = np.zeros((NVAR, W_XPAD), np.float32)
        lo = tau0 - (K - 2)  # first x column needed
        src_lo = max(lo, 0)
        xpad[:, src_lo - lo:W_XPAD - 1] = x[b, :, src_lo:tau0 + L + 1]
        a = dyn_weights[:, b, tau0:tau0 + L, :]          # [E, L, K]
        a = a.reshape(E, NCHUNK, CHUNK, K)               # [E, h2, tau, k]
        a = a.transpose(1, 0, 3, 2)                      # [h2, E, k, tau]
        dw = np.ascontiguousarray(a).reshape(NCHUNK * E, CHUNK * K)
        in_maps.append({
            "xpad": xpad,
            "dw": dw,
            "ssend": ssend,
            "wt": wt,
            "recvT": recvT,
            "bias_ones": bias_ones,
        })
    return in_maps


def _assemble(results):
    out = np.empty((B, TAU, NVAR, 1), np.float32)
    for core in range(NC_COUNT):
        b, h = divmod(core, 2)
        yT = results[core]["yT"]  # [NVAR, L]
        if h == 0:
            out[b, 0:L, :, 0] = yT.T
        else:
            out[b, L:TAU, :, 0] = yT[:, 1:L].T
    return out


def run_on_hw(in_maps, trace=False, **kwargs):
    from concourse.bass_utils import run_bass_kernel_spmd

    nc = _get_program()
    return run_bass_kernel_spmd(
        nc, in_maps, core_ids=list(range(NC_COUNT)), trace=trace, **kwargs
    )


def kernel(spikes, conv_weight, conv_bias, dyn_weights, edge_send, edge_recv):
    in_maps = _host_prep(
        spikes, conv_weight, conv_bias, dyn_weights, edge_send, edge_recv
    )
    res = run_on_hw(in_maps)
    return _assemble(res.results)



# revision 2
# speedup vs baseline: 1.9325x; 1.9325x over previous
"""Trainium2 Bass kernel for the spike-decoder GNN message-passing module.

Math (per batch b, output time tau in [0, T-2], variable v):
  out[b,tau,v] = bias[v]
               + sum_{i,k} w[v,i,k] * x[b,i,tau+k-(K-2)]          (static conv)
               + sum_{e: recv[e]=v} sum_k dw[e,b,tau,k] * x[b,send[e],tau+k-(K-2)]
with w = conv_weight masked at w[i,i,K-1] = 0, x = spikes[...,0] transposed to
[b, nvar, t], and out-of-range x treated as zero.

Sharding: 8 cores = (b in 0..3) x (time half h in 0..1). Each core computes a
1024-wide tau window ([0,1024) or [1023,2047) — one overlapping column keeps
shapes uniform for SPMD).

dyn_weights is the only big tensor; it streams as bf16 (exact relative to the
fp32 kernel: x is 0/1, so the masked products are bf16(dw) either way, and the
PE already consumed bf16 products). Halving the stream halves the DMA
bottleneck: ~18.6 MB/core at ~358 GB/s.

On-core algorithm:
  - xg[e,:] = x[send[e],:] gathered via one-hot matmul on PE (exact: x is 0/1),
    kept in two bf16 copies (xgA, xgB = xgA shifted left 1) so every sliding
    window the DVE reads starts 4B-aligned — that keeps tensor_tensor in its
    2x bf16 perf mode (odd-k windows would otherwise be 2-byte aligned -> 1x).
  - products P[e,(k,tau)] = dwt * window(xg) on DVE: per tile two bf16
    tensor_mul ops (even k's from xgA, odd k's from xgB), 3D APs stride-1 in
    tau.
  - k-reduction + recv-scatter + transpose folded into PE: for each k, a bf16
    matmul with stationary one-hot recv matrix and moving operand = P's k-slice,
    accumulating into PSUM[v, tau]
  - static conv: 16 bf16 matmuls with stationary wT_k and shifted xpad slices
    (xpadB = shifted copy keeps odd-k moving operands 4B-aligned)
  - bias: rank-1 matmul (bias x ones)
All terms accumulate into one PSUM bank [v, 512], copied out by ScalarE.
Output is [v, tau] per core; host transposes while assembling the result.
"""

import numpy as np

B, T, NVAR, K, E = 4, 2048, 128, 16, 512
TAU = T - 1            # 2047
L = 1024               # per-core tau window
NC_COUNT = 8
W_XPAD = L + K         # 1040 (1039 used; padded even)
ETILES = E // 128      # 4
CHUNK = 512            # tau chunk per PSUM bank
NCHUNK = L // CHUNK    # 2

_PROGRAM = None


def _build_program():
    import concourse.bass as bass
    import concourse.bacc as bacc
    import concourse.mybir as mybir
    import concourse.tile as tile

    f32 = mybir.dt.float32
    f32r = mybir.dt.float32r
    bf16 = mybir.dt.bfloat16
    # Bacc (not plain Bass): its compile pipeline runs generate_event_semaphores,
    # which splits multi-semaphore waits — a raw fp32 Matmult supports only one
    # sync-wait slot and walrus rejects more ("Too many sync wait commands").
    nc = bacc.Bacc()

    xpad_d = nc.declare_dram_parameter("xpad", [NVAR, 2 * W_XPAD], bf16, isOutput=False)
    dw_d = nc.declare_dram_parameter("dw", [NCHUNK * E, CHUNK * K], bf16, isOutput=False)
    ssend_d = nc.declare_dram_parameter("ssend", [NVAR, E], bf16, isOutput=False)
    wt_d = nc.declare_dram_parameter("wt", [NVAR, K * NVAR], bf16, isOutput=False)
    recv_d = nc.declare_dram_parameter("recvT", [128, ETILES * NVAR], bf16, isOutput=False)
    bo_d = nc.declare_dram_parameter("bias_ones", [1, NVAR + CHUNK], f32r, isOutput=False)
    y_d = nc.declare_dram_parameter("yT", [NVAR, L], f32, isOutput=True)

    with tile.TileContext(nc) as tc:
        with (
            tc.tile_pool(name="consts", bufs=1) as consts,
            tc.tile_pool(name="xgp", bufs=1) as xgp,
            tc.tile_pool(name="gpsum", bufs=2, space=bass.MemorySpace.PSUM) as gpsum,
            tc.tile_pool(name="dwp", bufs=3) as dwp,
            tc.tile_pool(name="prodp", bufs=3) as prodp,
            tc.tile_pool(name="opsum", bufs=2, space=bass.MemorySpace.PSUM) as opsum,
            tc.tile_pool(name="resp", bufs=2) as resp,
        ):
            NT = NCHUNK * ETILES  # 8 dw tiles
            HK = CHUNK * K // 2   # half-tile product columns (4096)

            # SP/HWDGE issue order = completion order (per-engine FIFO):
            # gather inputs first (small), then the dw stream owns the queue.
            # The tail tile is split into half-DMAs so its second half's
            # compute overlaps the first half's.
            xpad = consts.tile([NVAR, 2 * W_XPAD], bf16)
            nc.sync.dma_start(xpad[:], xpad_d[:])
            ssend = consts.tile([NVAR, E], bf16)
            nc.sync.dma_start(ssend[:], ssend_d[:])

            def dw_dma(dwt, ti, halves):
                h2, et = divmod(ti, ETILES)
                r0 = h2 * E + et * 128
                if halves:
                    for half in range(2):
                        nc.sync.dma_start(
                            dwt[:, half * HK:(half + 1) * HK],
                            dw_d[r0:r0 + 128, half * HK:(half + 1) * HK],
                        )
                else:
                    nc.sync.dma_start(dwt[:], dw_d[r0:r0 + 128, :])

            dwt_tiles = []
            for ti in range(NT):
                dwt = dwp.tile([128, CHUNK * K], bf16, name="dwt", tag="dwt")
                dwt_tiles.append(dwt)
            dw_dma(dwt_tiles[0], 0, halves=False)
            # remaining small constants slot in behind the first dw tile
            wt = consts.tile([NVAR, K * NVAR], bf16)
            nc.sync.dma_start(wt[:], wt_d[:])
            recvT = consts.tile([128, ETILES * NVAR], bf16)
            nc.sync.dma_start(recvT[:], recv_d[:])
            bias_ones = consts.tile([1, NVAR + CHUNK], f32r)
            nc.sync.dma_start(bias_ones[:], bo_d[:])
            for ti in range(1, NT):
                dw_dma(dwt_tiles[ti], ti, halves=(ti == NT - 1))

            # Gather sender rows: xgA[et][p, j] = xpad[send[et*128+p], j],
            # xgB = xgA shifted left by one column (for 4B-aligned odd-k
            # windows). A-copies from PSUM on ScalarE; B-copy SBUF->SBUF.
            xgA, xgB = [], []
            for et in range(ETILES):
                xga = xgp.tile([128, W_XPAD], bf16, name=f"xga{et}", tag=f"xga{et}")
                xgb = xgp.tile([128, W_XPAD], bf16, name=f"xgb{et}", tag=f"xgb{et}")
                for j0 in range(0, W_XPAD, CHUNK):
                    jw = min(CHUNK, W_XPAD - j0)
                    gps = gpsum.tile([128, CHUNK], f32, name="gps", tag="gps")
                    nc.tensor.matmul(
                        gps[:, :jw],
                        ssend[:, et * 128:(et + 1) * 128],
                        xpad[:, j0:j0 + jw],
                        start=True, stop=True,
                    )
                    nc.scalar.copy(xga[:, j0:j0 + jw], gps[:, :jw])
                nc.scalar.copy(xgb[:, 0:W_XPAD - 1], xga[:, 1:W_XPAD])
                xgA.append(xga)
                xgB.append(xgb)

            ops_tiles = []
            for h2 in range(NCHUNK):
                o = opsum.tile([128, CHUNK], f32, name=f"ops{h2}", tag=f"ops{h2}")
                ops_tiles.append(o)

            def static_mm(h2, k, start=False):
                t0 = h2 * CHUNK
                # odd k reads the shifted copy so the bf16 moving operand
                # stays 4B-aligned
                off = W_XPAD + t0 + k - 1 if (k % 2) else t0 + k
                nc.tensor.matmul(
                    ops_tiles[h2][:],
                    wt[:, k * NVAR:(k + 1) * NVAR],
                    xpad[:, off:off + CHUNK],
                    start=start, stop=False,
                )

            def bias_mm(h2):
                nc.tensor.matmul(
                    ops_tiles[h2][:],
                    bias_ones[:1, 0:NVAR],
                    bias_ones[:1, NVAR:NVAR + CHUNK],
                    start=False, stop=False,
                )

            # chunk-0 static conv + bias up front (PE warmup while dw streams)
            for k in range(K):
                static_mm(0, k, start=(k == 0))
            bias_mm(0)

            # chunk-1 static matmuls fill PE gaps across the first groups
            fill = [("s", k) for k in range(K)] + [("b", None)]
            fills_per_group = [3, 3, 3, 2, 2, 2, 2, 0, 0]

            def do_ks(h2, et, dwt, pt, ks):
                """One DVE tensor_mul over the k-set `ks` (uniform step 2),
                then the PE scatter matmuls for those k's."""
                t0 = h2 * CHUNK
                drow = dwt.tensor.shape[-1]
                prow = pt.tensor.shape[-1]
                par = ks[0] % 2  # 0 -> xgA, 1 -> xgB
                xg = xgB[et] if par else xgA[et]
                xrow = xg.tensor.shape[-1]
                nk = len(ks)
                in0 = bass.AP(dwt.tensor, ks[0] * CHUNK,
                              [[drow, 128], [2 * CHUNK, nk], [1, CHUNK]])
                # window: in1[p, j, tau] = xg[p, t0 + ks[0]+2j - par + tau]
                in1 = bass.AP(xg.tensor, t0 + ks[0] - par,
                              [[xrow, 128], [2, nk], [1, CHUNK]])
                out3 = bass.AP(pt.tensor, ks[0] * CHUNK,
                               [[prow, 128], [2 * CHUNK, nk], [1, CHUNK]])
                nc.vector.tensor_mul(out3, in0, in1)
                for k in ks:
                    rhs = bass.AP(pt.tensor, k * CHUNK,
                                  [[prow, 128], [1, CHUNK]])
                    nc.tensor.matmul(
                        ops_tiles[h2][:],
                        recvT[:, et * NVAR:(et + 1) * NVAR],
                        rhs,
                        start=False,
                        stop=(et == ETILES - 1 and k == K - 1),
                    )

            # groups: tiles 0..6 whole (evens then odds); tile 7 split into
            # k-halves so its second half-DMA overlaps the first's compute.
            groups = []
            for ti in range(NT - 1):
                groups.append((ti, [list(range(0, K, 2)), list(range(1, K, 2))]))
            groups.append((NT - 1, [list(range(0, K // 2, 2)), list(range(1, K // 2, 2))]))
            groups.append((NT - 1, [list(range(K // 2, K, 2)), list(range(K // 2 + 1, K, 2))]))

            pts = {}
            for gi, (ti, ksets) in enumerate(groups):
                h2, et = divmod(ti, ETILES)
                dwt = dwt_tiles[ti]
                if ti not in pts:
                    pts[ti] = prodp.tile([128, CHUNK * K], bf16, name="pt", tag="pt")
                pt = pts[ti]
                for ks in ksets:
                    do_ks(h2, et, dwt, pt, ks)
                for _ in range(fills_per_group[gi]):
                    kind, k = fill.pop(0)
                    if kind == "s":
                        static_mm(1, k, start=(k == 0))
                    else:
                        bias_mm(1)
                if et == ETILES - 1 and (gi == len(groups) - 1 or ti != NT - 1):
                    t0 = h2 * CHUNK
                    res = resp.tile([128, CHUNK], f32, name="res", tag="res")
                    nc.scalar.copy(res[:], ops_tiles[h2][:])
                    nc.gpsimd.dma_start(y_d[:, t0:t0 + CHUNK], res[:])

    nc.compile()
    return nc


def _get_program():
    global _PROGRAM
    if _PROGRAM is None:
        _PROGRAM = _build_program()
    return _PROGRAM


def _host_prep(spikes, conv_weight, conv_bias, dyn_weights, edge_send, edge_recv):
    import ml_dtypes
    bf16 = ml_dtypes.bfloat16

    spikes = np.asarray(spikes, dtype=np.float32)
    conv_weight = np.asarray(conv_weight, dtype=np.float32)
    conv_bias = np.asarray(conv_bias, dtype=np.float32)
    dyn_weights = np.asarray(dyn_weights)
    edge_send = np.asarray(edge_send, dtype=np.int64)
    edge_recv = np.asarray(edge_recv, dtype=np.int64)

    x = np.ascontiguousarray(spikes[..., 0].transpose(0, 2, 1))  # [B, NVAR, T]

    ssend = np.zeros((NVAR, E), bf16)
    ssend[edge_send, np.arange(E)] = 1.0

    recvT = np.zeros((128, ETILES * NVAR), bf16)
    for et in range(ETILES):
        rr = edge_recv[et * 128:(et + 1) * 128]
        recvT[np.arange(128), et * NVAR + rr] = 1.0

    w = conv_weight.copy()
    w[np.arange(NVAR), np.arange(NVAR), K - 1] = 0.0
    wt = np.ascontiguousarray(w.transpose(1, 2, 0)).reshape(NVAR, K * NVAR).astype(bf16)

    bias_ones = np.concatenate(
        [conv_bias, np.ones(CHUNK, np.float32)]
    ).reshape(1, NVAR + CHUNK).astype(np.float32)

    dwb = dyn_weights.astype(bf16)  # [E, B, T-1, K]

    in_maps = []
    for core in range(NC_COUNT):
        b, h = divmod(core, 2)
        tau0 = 0 if h == 0 else TAU - L  # 0 or 1023
        xpad2 = np.zeros((NVAR, 2 * W_XPAD), np.float32)
        lo = tau0 - (K - 2)  # first x column needed
        src_lo = max(lo, 0)
        xpad2[:, src_lo - lo:W_XPAD - 1] = x[b, :, src_lo:tau0 + L + 1]
        # second half = shifted-left-by-one copy (odd-k aligned windows)
        xpad2[:, W_XPAD:2 * W_XPAD - 1] = xpad2[:, 1:W_XPAD]
        a = dwb[:, b, tau0:tau0 + L, :]                  # [E, L, K]
        a = a.reshape(E, NCHUNK, CHUNK, K)               # [E, h2, tau, k]
        a = a.transpose(1, 0, 3, 2)                      # [h2, E, k, tau]
        dw = np.ascontiguousarray(a).reshape(NCHUNK * E, CHUNK * K)
        in_maps.append({
            "xpad": xpad2.astype(bf16),
            "dw": dw,
            "ssend": ssend,
            "wt": wt,
            "recvT": recvT,
            "bias_ones": bias_ones,
        })
    return in_maps


def _assemble(results):
    out = np.empty((B, TAU, NVAR, 1), np.float32)
    for core in range(NC_COUNT):
        b, h = divmod(core, 2)
        yT = results[core]["yT"]  # [NVAR, L]
        if h == 0:
            out[b, 0:L, :, 0] = yT.T
        else:
            out[b, L:TAU, :, 0] = yT[:, 1:L].T
    return out


def run_on_hw(in_maps, trace=False, **kwargs):
    from concourse.bass_utils import run_bass_kernel_spmd

    nc = _get_program()
    return run_bass_kernel_spmd(
        nc, in_maps, core_ids=list(range(NC_COUNT)), trace=trace, **kwargs
    )


def kernel(spikes, conv_weight, conv_bias, dyn_weights, edge_send, edge_recv):
    in_maps = _host_prep(
        spikes, conv_weight, conv_bias, dyn_weights, edge_send, edge_recv
    )
    res = run_on_hw(in_maps)
    return _assemble(res.results)
